# revision 7
# baseline (speedup 1.0000x reference)
"""GCNConv Trainium2 kernel: out = relu((A @ (X @ W)) + bias).

Strategy (8 NeuronCores, SPMD single program):
  - Host: balance destination rows into 784 windows of 128 dests each
    (serpentine deal by degree, so every window has ~mean edge count),
    assign 98 windows per core, group each core's edges into windows,
    pad every window to a uniform tile count so all cores run the
    identical program.  x is cast to bf16 (halves gather traffic).
  - Device (per core): agg = A_c @ X via per-edge indirect-DMA gather of
    bf16 x rows + "val-hot" selection-matrix matmuls accumulating in
    PSUM (segment-sum as one-hot matmul); then out_c = relu(agg @ W + b)
    using PE transposes of agg (matmul associativity: A@(XW) == (A@X)@W,
    so the dense feature transform happens once per output row, not per
    edge).  Finalize stays f32 for accuracy.

Constants ship in three DRAM tensors (f32 header, bf16 plane, i32 cols),
one DMA each, so consumers wait on few DMA semaphores (walrus rejects
instructions with too many sync waits; _legalize_waits hoists the rest).
"""

import math
import sys
from contextlib import ExitStack

import numpy as np
import ml_dtypes

sys.path.insert(0, "/opt/trn_rl_repo")

import concourse.bass as bass
import concourse.tile as tile
from concourse import mybir
from concourse.bass_utils import run_bass_kernel_spmd

F32 = mybir.dt.float32
BF16 = mybir.dt.bfloat16
I32 = mybir.dt.int32
NP_BF16 = ml_dtypes.bfloat16

N_NODES = 100000
N_EDGES = 3200000
D_FEAT = 256
UNITS = 256
NCORES = 8
W = 128                          # destination window width (= PSUM partitions)
NW = 98                          # windows per core (98*128 = 12544 >= 12500)
NW_G = NCORES * NW               # 784 global windows
NPC = NW * W                     # output rows per core (12544, some padding)

# f32 consts layout (free-dim offsets in the [128, CFH + 2*nt] f32 tensor):
#   identity [0:128] | w0 [128:384] | w1 [384:640] | bias [640:896]
#   | vals [896:896+nt] | drel [896+nt:896+2nt]
# (tensor_scalar requires f32 scalar operands, so vals/drel stay f32)
CFH = 128 + 256 + 256 + 256      # 896
# bf16 plane: iota [0:128] only (in0/out bf16 for 2x DVE rate)

LAST_RESULTS = None
LAST_IN_MAPS = None
LAST_NC = None

_NC_CACHE = {}

_WAIT_EXEMPT = {"InstEventSemaphore"}


def _legalize_waits(nc, limit=1):
    """Walrus allows very few sync waits per compute/DMA instruction (the
    LDWEIGHTS/TS structs take one).  Hoist excess waits onto standalone
    InstEventSemaphore instructions placed just before, on the same engine
    queue.  Each carrier gets an update on a dummy semaphore (the race
    detector requires every executable instruction to update something)."""
    used = set()
    for fn in nc.m.functions:
        for blk in fn.blocks:
            for inst in blk.instructions:
                si = inst.sync_info
                if si is None:
                    continue
                for wt in si.on_wait:
                    used.add(wt.id)
                for up in si.on_update:
                    used.add(up.id)
    sem_range = bass.get_kernel_semaphore_range()
    free = [i for i in sem_range if i not in used]
    assert free, "no free semaphore for wait legalization"
    dummy_num = free[-1]
    n_hoisted = 0
    for fn in nc.m.functions:
        for blk in fn.blocks:
            insts = blk.instructions
            out = []
            changed = False
            for inst in insts:
                si = inst.sync_info
                tname = type(inst).__name__
                if (si is not None and tname not in _WAIT_EXEMPT
                        and len(si.on_wait) > limit):
                    waits = list(si.on_wait)
                    # Keep compute-engine waits on the instruction itself
                    # (walrus attaches them to the first uop, e.g. LDWEIGHTS,
                    # which the PE may pull ahead of queued predecessors);
                    # hoist DMA-lane waits onto the EVSEM carrier.
                    waits.sort(key=lambda w: (w.ant_name or "").startswith("DMA"))
                    waits.reverse()  # DMA waits first (hoisted), engine last
                    for j, wt in enumerate(waits[:-limit]):
                        out.append(mybir.InstEventSemaphore(
                            name=f"{inst.name}-hw{j}",
                            engine=inst.engine,
                            ins=[],
                            outs=[],
                            sync_info=mybir.SyncInfo(
                                on_wait=[wt],
                                on_update=[mybir.SyncUpdate(
                                    sync_type="semaphore",
                                    id=dummy_num,
                                    ant_name="legalize_dummy",
                                    update_mode="sem-inc",
                                    update_value=1)]),
                        ))
                        n_hoisted += 1
                    inst.sync_info = mybir.SyncInfo(
                        on_wait=waits[-limit:],
                        on_update=list(si.on_update))
                    changed = True
                out.append(inst)
            if changed:
                blk.instructions = out
    return n_hoisted


def build_nc(t_w, gather_k, n_nodes=N_NODES, d_feat=D_FEAT, units=UNITS):
    """Build the SPMD Bass program (identical on all 8 cores)."""
    nw = NW
    nt = nw * t_w                    # total edge tiles per core
    cff = CFH + 2 * nt               # f32 consts free dim

    nc = bass.Bass("TRN2", target_bir_lowering=False, debug=False,
                   num_devices=NCORES)

    x = nc.dram_tensor("x", [n_nodes, d_feat], BF16, kind="ExternalInput")
    cF_d = nc.dram_tensor("constsF", [128, cff], F32, kind="ExternalInput")
    cB_d = nc.dram_tensor("constsB", [128, 128], BF16, kind="ExternalInput")
    cols_d = nc.dram_tensor("cols", [128, nt], I32, kind="ExternalInput")
    out_d = nc.dram_tensor("out", [nw * W, units], F32, kind="ExternalOutput")

    with tile.TileContext(nc) as tc, ExitStack() as ctx:
        const = ctx.enter_context(tc.tile_pool(name="const", bufs=1))
        msgs_p = ctx.enter_context(tc.tile_pool(name="msgs", bufs=4))
        vh_p = ctx.enter_context(tc.tile_pool(name="vh", bufs=8))
        agg_p = ctx.enter_context(tc.tile_pool(name="agg", bufs=3))
        aggT_p = ctx.enter_context(tc.tile_pool(name="aggT", bufs=4))
        out_p = ctx.enter_context(tc.tile_pool(name="outp", bufs=3))
        ps_agg = ctx.enter_context(tc.tile_pool(name="ps_agg", bufs=2, space="PSUM"))
        ps_tp = ctx.enter_context(tc.tile_pool(name="ps_tp", bufs=2, space="PSUM"))
        ps_out = ctx.enter_context(tc.tile_pool(name="ps_out", bufs=2, space="PSUM"))

        cF = const.tile([128, cff], F32)
        nc.sync.dma_start(cF[:], cF_d[:])
        cB = const.tile([128, 128], BF16)
        nc.sync.dma_start(cB[:], cB_d[:])
        cols_s = const.tile([128, nt], I32)
        nc.sync.dma_start(cols_s[:], cols_d[:])

        identity = cF[:, 0:128]
        wt = [cF[:, 128:384], cF[:, 384:640]]
        bias_s = cF[:, 640:896]
        vals_s = cF[:, CFH:CFH + nt]
        drel_s = cF[:, CFH + nt:CFH + 2 * nt]
        iota_s = cB[:, 0:128]

        ngroups = t_w // gather_k

        for w in range(nw):
            agg_ps = ps_agg.tile([128, d_feat], F32)
            for g in range(ngroups):
                msgs = msgs_p.tile([128, gather_k * d_feat], BF16)
                t0 = w * t_w + g * gather_k
                nc.gpsimd.indirect_dma_start(
                    out=msgs[:],
                    out_offset=None,
                    in_=x[:],
                    in_offset=bass.IndirectOffsetOnAxis(
                        ap=cols_s[:, t0:t0 + gather_k], axis=0),
                )
                for j in range(gather_k):
                    t = g * gather_k + j
                    ti = w * t_w + t
                    vh = vh_p.tile([128, W], BF16)
                    # vh[p, m] = (iota[m] == drel[p]) * val[p]
                    nc.vector.tensor_scalar(
                        out=vh[:],
                        in0=iota_s,
                        scalar1=drel_s[:, ti:ti + 1],
                        scalar2=vals_s[:, ti:ti + 1],
                        op0=mybir.AluOpType.is_equal,
                        op1=mybir.AluOpType.mult,
                    )
                    # agg[dest, feat] += vh.T @ msgs_tile
                    nc.tensor.matmul(
                        agg_ps[:],
                        lhsT=vh[:],
                        rhs=msgs[:, j * d_feat:(j + 1) * d_feat],
                        start=(t == 0),
                        stop=(t == t_w - 1),
                    )
            # Finalize window: out_win = relu(agg @ W + bias)  (f32)
            agg_s = agg_p.tile([128, d_feat], F32)
            nc.vector.tensor_copy(agg_s[:], agg_ps[:])
            out_ps = ps_out.tile([128, units], F32)
            for kh in range(d_feat // 128):
                tp_ps = ps_tp.tile([128, 128], F32)
                nc.tensor.transpose(
                    tp_ps[:], agg_s[:, kh * 128:(kh + 1) * 128], identity)
                aggT = aggT_p.tile([128, 128], F32)
                nc.vector.tensor_copy(aggT[:], tp_ps[:])
                nc.tensor.matmul(
                    out_ps[:],
                    lhsT=aggT[:],
                    rhs=wt[kh],
                    start=(kh == 0),
                    stop=(kh == d_feat // 128 - 1),
                )
            out_s = out_p.tile([128, units], F32)
            nc.vector.tensor_tensor(
                out=out_s[:], in0=out_ps[:], in1=bias_s,
                op=mybir.AluOpType.add)
            nc.vector.tensor_scalar_max(out_s[:], out_s[:], 0.0)
            nc.sync.dma_start(out_d[w * 128:(w + 1) * 128, :], out_s[:])

    _legalize_waits(nc)
    return nc


def _pack_windows(edge_row, n_nodes=N_NODES):
    """Assign each destination row to a (global window, slot) so that all
    784 windows have near-equal edge counts.  Serpentine deal of dests in
    descending-degree order; returns (win_of, slot_of, dest_of_outrow)."""
    deg = np.bincount(edge_row, minlength=n_nodes)
    order = np.argsort(-deg, kind="stable").astype(np.int64)
    total_slots = NW_G * W
    padded = np.full(total_slots, -1, np.int64)
    padded[:n_nodes] = order
    rounds = padded.reshape(W, NW_G).copy()
    rounds[1::2] = rounds[1::2, ::-1]           # serpentine
    win_of = np.empty(n_nodes, np.int32)
    slot_of = np.empty(n_nodes, np.int32)
    valid = rounds >= 0
    win_idx = np.broadcast_to(np.arange(NW_G, dtype=np.int32)[None, :],
                              rounds.shape)
    slot_idx = np.broadcast_to(np.arange(W, dtype=np.int32)[:, None],
                               rounds.shape)
    win_of[rounds[valid]] = win_idx[valid]
    slot_of[rounds[valid]] = slot_idx[valid]

    # Repair pass: cap the max window load near the mean so t_w stays at
    # ceil(mean/128).  Swap the highest-degree dest of the heaviest window
    # with a low-degree dest of the lightest window until within target.
    wsum = np.bincount(win_of[edge_row], minlength=NW_G).astype(np.int64)
    target = max(int(np.ceil(wsum.mean())) + 8,
                 128 * int(np.ceil(wsum.mean() / 128)))
    if wsum.max() > target:
        win_members = [[] for _ in range(NW_G)]
        for d in range(n_nodes):
            win_members[win_of[d]].append(d)
        for _ in range(20000):
            hi = int(np.argmax(wsum))
            if wsum[hi] <= target:
                break
            lo = int(np.argmin(wsum))
            mh = win_members[hi]
            ml = win_members[lo]
            dh = max(mh, key=lambda d: deg[d])
            dl = min(ml, key=lambda d: deg[d])
            if deg[dh] <= deg[dl]:
                break
            mh[mh.index(dh)] = dl
            ml[ml.index(dl)] = dh
            delta = int(deg[dh] - deg[dl])
            wsum[hi] -= delta
            wsum[lo] += delta
            sh, sl = slot_of[dh], slot_of[dl]
            win_of[dh], slot_of[dh] = lo, sl
            win_of[dl], slot_of[dl] = hi, sh

    # dest_of_outrow: global output row (win*128 + slot) -> dest (or -1)
    dest_of_outrow = np.full(total_slots, -1, np.int64)
    dest_of_outrow[win_of.astype(np.int64) * W + slot_of] = np.arange(n_nodes)
    return win_of, slot_of, dest_of_outrow


def prep_inputs(edge_row, edge_col, edge_val, x, weight, bias):
    """Host-side sharding: balance dests into windows, sort edges by
    window, build per-core padded planes + packed consts."""
    edge_row = np.ascontiguousarray(edge_row)
    edge_col = np.ascontiguousarray(edge_col)
    edge_val = np.ascontiguousarray(edge_val, dtype=np.float32)
    x = np.ascontiguousarray(x, dtype=np.float32)
    weight = np.ascontiguousarray(weight, dtype=np.float32)
    bias = np.ascontiguousarray(bias, dtype=np.float32)

    win_of, slot_of, dest_of_outrow = _pack_windows(edge_row)

    wglob = win_of[edge_row]                      # global window per edge
    drel = slot_of[edge_row].astype(np.float32)   # slot-in-window per edge

    order = np.argsort(wglob, kind="stable")
    counts = np.bincount(wglob, minlength=NW_G)

    t_w_raw = int(math.ceil(counts.max() / W))
    t_w = max(t_w_raw, 32)
    gather_k = 1
    slots = t_w * W
    nt = NW * t_w

    s_col = edge_col[order]
    s_val = edge_val[order]
    s_drel = drel[order]

    # position of each (sorted) edge within its window
    offs = np.zeros(NW_G + 1, np.int64)
    np.cumsum(counts, out=offs[1:])
    pos = np.arange(len(s_col), dtype=np.int64) - np.repeat(offs[:-1], counts)

    buf_c = np.zeros((NW_G, slots), np.int32)
    buf_v = np.zeros((NW_G, slots), np.float32)
    buf_d = np.zeros((NW_G, slots), np.float32)
    flat = wglob[order] * slots + pos
    buf_c.ravel()[flat] = s_col
    buf_v.ravel()[flat] = s_val
    buf_d.ravel()[flat] = s_drel

    # [NW_G, slots] -> per core [128, nw*t_w] planes; tile ti = w*t_w + t
    # holds edges [t*128, (t+1)*128) of window w on partitions.
    def core_planes(buf, dtype):
        a = buf.reshape(NCORES, NW, t_w, W).transpose(0, 3, 1, 2)
        return np.ascontiguousarray(a.reshape(NCORES, W, NW * t_w).astype(dtype))

    cols_h = core_planes(buf_c, np.int32)
    vals_h = core_planes(buf_v, np.float32)
    drel_h = core_planes(buf_d, np.float32)

    x_bf = x.astype(NP_BF16)

    hdrF = np.zeros((128, CFH), np.float32)
    hdrF[:, 0:128] = np.eye(128, dtype=np.float32)
    hdrF[:, 128:384] = weight[0:128, :]
    hdrF[:, 384:640] = weight[128:256, :]
    hdrF[:, 640:896] = bias[None, :]

    iota_b = np.ascontiguousarray(np.broadcast_to(
        np.arange(W, dtype=np.float32)[None, :], (128, W)).astype(NP_BF16))

    in_maps = []
    for c in range(NCORES):
        constsF = np.concatenate([hdrF, vals_h[c], drel_h[c]], axis=1)
        in_maps.append({
            "x": x_bf,
            "constsF": np.ascontiguousarray(constsF),
            "constsB": iota_b,
            "cols": cols_h[c],
        })
    return in_maps, t_w, gather_k, dest_of_outrow


def kernel(edge_row, edge_col, edge_val, x, weight, bias, **run_kwargs):
    global LAST_RESULTS, LAST_IN_MAPS, LAST_NC
    in_maps, t_w, gather_k, dest_of_outrow = prep_inputs(
        edge_row, edge_col, edge_val, x, weight, bias)
    key = (t_w, gather_k)
    if key not in _NC_CACHE:
        _NC_CACHE[key] = build_nc(t_w=t_w, gather_k=gather_k)
    nc = _NC_CACHE[key]
    res = run_bass_kernel_spmd(nc, in_maps, core_ids=list(range(NCORES)),
                               **run_kwargs)
    LAST_RESULTS = res
    LAST_IN_MAPS = in_maps
    LAST_NC = nc
    flat = np.concatenate([res.results[c]["out"] for c in range(NCORES)],
                          axis=0)                  # [NW_G*128, units]
    out = np.empty((N_NODES, UNITS), np.float32)
    rows = dest_of_outrow
    sel = rows >= 0
    out[rows[sel]] = flat[sel]
    return out
